# revision 2
# baseline (speedup 1.0000x reference)
"""3-branch GCN (DGL GraphConv x3 + max-pool + MLP head) on 8 TRN2 NeuronCores.

Sharding: destination nodes (2500/core). Per layer, each core gathers src rows
from a replicated DRAM table (per-chunk indirect DMA), aggregates via one-hot
fp16 matmuls into PSUM, applies the dense W matmul per dst tile, and the layer
output shards are AllGathered for the next layer. Max-pool is local + a final
AllReduce(max); the tiny MLP head runs replicated on every core.
"""
import numpy as np
import concourse.bass as bass
import concourse.bacc as bacc
import concourse.tile as tile
import concourse.mybir as mybir
from concourse.bass_utils import run_bass_kernel_spmd

NC_ = 8
N = 20000
E = 320000
SH = N // NC_          # 2500 nodes per core
NT = 20                # dst tiles per core (19 full + 68-node partial)
D_IN, D_H = 128, 304
DPAD = 384             # fp16 row pad for 768B (256B-mult) rows
f16, f32 = mybir.dt.float16, mybir.dt.float32
i32 = mybir.dt.int32
AF = mybir.ActivationFunctionType
core_ids = list(range(NC_))


def _prep_graph(src, dst):
    """Per-core chunked edge metadata with core-uniform chunk counts."""
    src = np.asarray(src).astype(np.int64)
    dst = np.asarray(dst).astype(np.int64)
    outdeg = np.bincount(src, minlength=N).clip(1).astype(np.float32)
    indeg = np.bincount(dst, minlength=N).clip(1).astype(np.float32)
    per_core = []
    for c in range(NC_):
        m = (dst // SH) == c
        es, ed = src[m], dst[m] - c * SH
        tiles = []
        for t in range(NT):
            tm = (ed // 128) == t
            tiles.append((es[tm], ed[tm] - t * 128))
        per_core.append(tiles)
    # uniform chunk count per tile slot
    Ck = [max(int(np.ceil(len(per_core[c][t][0]) / 128)) for c in range(NC_)) or 1
          for t in range(NT)]
    nchunks = sum(Ck)
    offs = np.full((NC_, 128, nchunks), 0, np.int32)      # pad -> row 0 (S row zero)
    drel = np.full((NC_, 128, nchunks), -1.0, np.float32)  # -1 -> zero S row
    for c in range(NC_):
        j0 = 0
        for t in range(NT):
            es, er = per_core[c][t]
            npad = Ck[t] * 128
            e_s = np.full(npad, 0, np.int64)
            e_r = np.full(npad, -1.0, np.float32)
            e_s[:len(es)] = es
            e_r[:len(er)] = er
            offs[c, :, j0:j0 + Ck[t]] = e_s.reshape(Ck[t], 128).T
            drel[c, :, j0:j0 + Ck[t]] = e_r.reshape(Ck[t], 128).T
            j0 += Ck[t]
    # per-core slot-ordered degree arrays [128, NT]
    ind = np.ones((NC_, 128, NT), np.float32)
    outd = np.ones((NC_, 128, NT), np.float32)
    for c in range(NC_):
        for t in range(NT):
            lo = c * SH + t * 128
            hi = min(lo + 128, (c + 1) * SH)
            ind[c, :hi - lo, t] = indeg[lo:hi]
            outd[c, :hi - lo, t] = outdeg[lo:hi]
    # full outdeg [128, 157] node n -> [n%128, n//128] (lane-major tiles)
    odf = np.ones((128, 157), np.float32)
    odf.reshape(-1)[:0] = 0
    tmp = np.ones(157 * 128, np.float32)
    tmp[:N] = outdeg
    odf = tmp.reshape(157, 128).T.copy()
    return Ck, offs, drel, ind, outd, odf


def _build(g_meta):
    nc = bacc.Bacc(None, target_bir_lowering=False)
    ext = {}
    for g in range(3):
        Ck, offs, drel, ind, outd, odf = g_meta[g]
        nch = sum(Ck)
        ext[f"x{g}"] = nc.dram_tensor(f"x{g}", [N, D_IN], f32, kind="ExternalInput")
        ext[f"off{g}"] = nc.dram_tensor(f"off{g}", [128, nch], i32, kind="ExternalInput")
        ext[f"dr{g}"] = nc.dram_tensor(f"dr{g}", [128, nch], f32, kind="ExternalInput")
        ext[f"ind{g}"] = nc.dram_tensor(f"ind{g}", [128, NT], f32, kind="ExternalInput")
        ext[f"outd{g}"] = nc.dram_tensor(f"outd{g}", [128, NT], f32, kind="ExternalInput")
        ext[f"odf{g}"] = nc.dram_tensor(f"odf{g}", [128, 157], f32, kind="ExternalInput")
    for nm, shp in [("W1", [D_IN, D_H]), ("W2", [D_H, D_H]), ("W3", [D_H, D_H]),
                    ("b1", [1, D_H]), ("b2", [1, D_H]), ("b3", [1, D_H]),
                    ("fW1", [D_H, 128]), ("fb1", [1, 128]), ("fW2", [128, 64]),
                    ("fb2", [1, 64]), ("fW3", [64, 1]), ("fb3", [1, 1])]:
        ext[nm] = nc.dram_tensor(nm, shp, f32, kind="ExternalInput")
    y_ext = nc.dram_tensor("y", [1, 1], f32, kind="ExternalOutput")

    iota_d = nc.inline_tensor(np.tile(np.arange(128, dtype=np.float16), (128, 1)),
                              name="iota128")
    ident_d = nc.inline_tensor(np.eye(128, dtype=np.float32), name="ident")
    ones16_d = nc.inline_tensor(np.ones((1, 128), np.float16), name="ones16")
    ones32_d = nc.inline_tensor(np.ones((1, 1), np.float32), name="ones32")
    zero_d = nc.inline_tensor(np.zeros((1, DPAD), np.float16), name="zrow")

    with tile.TileContext(nc) as tc:
        with (
            tc.tile_pool(name="cst", bufs=1) as cst,
            tc.tile_pool(name="meta", bufs=1) as meta,
            tc.tile_pool(name="g", bufs=10) as gp,
            tc.tile_pool(name="s", bufs=4) as sp,
            tc.tile_pool(name="w", bufs=3) as wp,
            tc.tile_pool(name="ps", bufs=2, space="PSUM") as pp,
            tc.tile_pool(name="ps2", bufs=2, space="PSUM") as pp2,
            tc.tile_pool(name="dram", bufs=1, space="DRAM") as dram,
        ):
            iota_t = cst.tile([128, 128], f16)
            nc.sync.dma_start(iota_t[:], iota_d[:])
            ident_t = cst.tile([128, 128], f32)
            nc.sync.dma_start(ident_t[:], ident_d[:])
            ones16 = cst.tile([1, 128], f16)
            nc.sync.dma_start(ones16[:], ones16_d[:])
            ones32 = cst.tile([1, 1], f32)
            nc.sync.dma_start(ones32[:], ones32_d[:])
            zrow = cst.tile([1, DPAD], f16)
            nc.sync.dma_start(zrow[:], zero_d[:])

            # weights resident
            W_t = {}
            w1t = cst.tile([128, D_H], f16, name="w1t")
            W_t[1] = [w1t]
            nc.gpsimd.dma_start(W_t[1][0][:], ext["W1"][:])
            for L in (2, 3):
                W_t[L] = []
                for j in range(3):
                    k = 128 if j < 2 else 48
                    w = cst.tile([128, D_H], f16, name=f"w{L}_{j}")
                    nc.gpsimd.dma_start(w[0:k, :], ext[f"W{L}"][j * 128:j * 128 + k, :])
                    W_t[L].append(w)
            b_t = {}
            for L in (1, 2, 3):
                b = cst.tile([1, D_H], f16, name=f"b{L}t")
                nc.gpsimd.dma_start(b[:], ext[f"b{L}"][:])
                b_t[L] = b
            fW1_t = []
            for j in range(3):
                k = 128 if j < 2 else 48
                w = cst.tile([128, 128], f32, name=f"fw1_{j}")
                nc.sync.dma_start(w[0:k, :], ext["fW1"][j * 128:j * 128 + k, :])
                fW1_t.append(w)
            fW2_t = cst.tile([128, 64], f32)
            nc.sync.dma_start(fW2_t[:], ext["fW2"][:])
            fW3_t = cst.tile([64, 1], f32)
            nc.sync.dma_start(fW3_t[:], ext["fW3"][:])
            fb_t = {}
            for nm, w in [("fb1", 128), ("fb2", 64), ("fb3", 1)]:
                b = cst.tile([1, w], f32, name=f"{nm}t")
                nc.sync.dma_start(b[:], ext[nm][:])
                fb_t[nm] = b

            # DRAM feature tables
            xp = dram.tile([N, D_IN], f16)
            hfA, hfB = [], []
            for g in range(3):
                ta = dram.tile([N, DPAD], f16, addr_space="Shared", name=f"hfA{g}")
                tb = dram.tile([N, DPAD], f16, addr_space="Shared", name=f"hfB{g}")
                hfA.append(ta)
                hfB.append(tb)
            shard_b = dram.tile([SH, DPAD], f16)
            pool_in = dram.tile([128, 3], f32)
            pool_out = dram.tile([128, 3], f32, addr_space="Shared")
            vec_b = dram.tile([1, 128], f32)

            macc = cst.tile([128, D_H], f32)
            nc.vector.memset(macc[:], 0.0)

            for g in range(3):
                Ck, offs_np, drel_np, _, _, _ = g_meta[g]
                nch = sum(Ck)
                off_t = meta.tile([128, nch], i32, tag="off")
                nc.sync.dma_start(off_t[:], ext[f"off{g}"][:])
                dr_t = meta.tile([128, nch], f32, tag="dr")
                nc.sync.dma_start(dr_t[:], ext[f"dr{g}"][:])
                # degree rsqrt arrays
                rind = meta.tile([128, NT], f32, tag="rind")
                tmp = meta.tile([128, NT], f32, tag="tmpd")
                nc.sync.dma_start(tmp[:], ext[f"ind{g}"][:])
                nc.scalar.sqrt(rind[:], tmp[:])
                nc.vector.reciprocal(rind[:], rind[:])
                rout = meta.tile([128, NT], f32, tag="rout")
                tmp2 = meta.tile([128, NT], f32, tag="tmpd2")
                nc.sync.dma_start(tmp2[:], ext[f"outd{g}"][:])
                nc.scalar.sqrt(rout[:], tmp2[:])
                nc.vector.reciprocal(rout[:], rout[:])
                rodf = meta.tile([128, 157], f32, tag="rodf")
                tmp3 = meta.tile([128, 157], f32, tag="tmpd3")
                nc.sync.dma_start(tmp3[:], ext[f"odf{g}"][:])
                nc.scalar.sqrt(rodf[:], tmp3[:])
                nc.vector.reciprocal(rodf[:], rodf[:])

                # x prescale -> xp (fp16)
                for t in range(157):
                    rows = 128 if t < 156 else N - 156 * 128
                    xt = gp.tile([128, D_IN], f32, tag="xt")
                    nc.sync.dma_start(xt[0:rows, :], ext[f"x{g}"][t * 128:t * 128 + rows, :])
                    xs = gp.tile([128, D_IN], f16, tag="xs")
                    nc.scalar.activation(xs[0:rows, :], xt[0:rows, :], AF.Copy,
                                         scale=rodf[0:rows, t:t + 1])
                    nc.sync.dma_start(xp[t * 128:t * 128 + rows, :], xs[0:rows, :])

                for L in (1, 2, 3):
                    src_tab = xp if L == 1 else (hfA[g] if L == 2 else hfB[g])
                    DL = D_IN if L == 1 else D_H
                    DLP = D_IN if L == 1 else DPAD
                    J = 1 if L == 1 else 3
                    j0 = 0
                    for t in range(NT):
                        rows = 128 if t < NT - 1 else SH - (NT - 1) * 128
                        psum = pp.tile([128, D_H], f32, tag="agg")
                        for cchunk in range(Ck[t]):
                            gt = gp.tile([128, DLP], f16, tag=f"g{L}")
                            nc.gpsimd.indirect_dma_start(
                                out=gt[:], out_offset=None, in_=src_tab[:],
                                in_offset=bass.IndirectOffsetOnAxis(
                                    ap=off_t[:, j0 + cchunk:j0 + cchunk + 1], axis=0))
                            s = sp.tile([128, 128], f16, tag="s")
                            nc.vector.tensor_scalar(
                                s[:], iota_t[:], dr_t[:, j0 + cchunk:j0 + cchunk + 1],
                                None, mybir.AluOpType.is_equal)
                            nc.tensor.matmul(psum[:, 0:DL], s[:], gt[:, 0:DL],
                                             start=(cchunk == 0), stop=(cchunk == Ck[t] - 1))
                        j0 += Ck[t]
                        # scale by rsqrt(indeg), transpose, W matmul
                        zsb = gp.tile([128, D_H], f32, tag="zsb")
                        nc.scalar.activation(zsb[:, 0:DL], psum[:, 0:DL], AF.Copy,
                                             scale=rind[:, t:t + 1])
                        psum2 = pp2.tile([128, D_H], f32, tag="wout")
                        for j in range(J):
                            k = 128 if (j < J - 1 or L == 1) else 48
                            tp = pp.tile([128, 128], f32, tag="tp")
                            nc.tensor.transpose(tp[0:k, :], zsb[:, j * 128:j * 128 + k],
                                                ident_t[:])
                            at = gp.tile([128, 128], f16, tag="at")
                            nc.vector.tensor_copy(at[0:k, :], tp[0:k, :])
                            nc.tensor.matmul(psum2[:], at[0:k, :], W_t[L][j][0:k, :],
                                             start=(j == 0), stop=False)
                        nc.tensor.matmul(psum2[:], ones16[:], b_t[L][:],
                                         start=False, stop=True)
                        if L < 3:
                            hsb = gp.tile([128, D_H], f16, tag="hsb")
                            nc.scalar.activation(hsb[:], psum2[:], AF.Relu,
                                                 scale=rout[:, t:t + 1])
                            nc.sync.dma_start(
                                shard_b[t * 128:t * 128 + rows, 0:D_H], hsb[0:rows, :])
                        else:
                            hsb = gp.tile([128, D_H], f32, tag="hsb3")
                            nc.scalar.activation(hsb[:], psum2[:], AF.Relu)
                            nc.vector.tensor_tensor(macc[0:rows, :], macc[0:rows, :],
                                                    hsb[0:rows, :], mybir.AluOpType.max)
                    if L < 3:
                        dstf = hfA[g] if L == 1 else hfB[g]
                        nc.gpsimd.collective_compute(
                            "AllGather", mybir.AluOpType.bypass,
                            replica_groups=[core_ids],
                            ins=[shard_b.opt()],
                            outs=[dstf.opt()])

            # max over partitions via transpose + reduce, AllReduce, MLP
            pool_sb = cst.tile([128, 3], f32)
            for j in range(3):
                k = 128 if j < 2 else 48
                tp = pp.tile([128, 128], f32, tag="tp")
                nc.tensor.transpose(tp[0:k, :], macc[:, j * 128:j * 128 + k], ident_t[:])
                nc.vector.tensor_reduce(pool_sb[0:k, j:j + 1], tp[0:k, :],
                                        mybir.AxisListType.X, mybir.AluOpType.max)
            nc.sync.dma_start(pool_in[:], pool_sb[:])
            nc.gpsimd.collective_compute(
                "AllReduce", mybir.AluOpType.max, replica_groups=[core_ids],
                ins=[pool_in.opt()], outs=[pool_out.opt()])
            pool_t = cst.tile([128, 3], f32)
            nc.sync.dma_start(pool_t[:], pool_out[:])

            z1p = pp2.tile([1, 128], f32, tag="z")
            for j in range(3):
                k = 128 if j < 2 else 48
                nc.tensor.matmul(z1p[:], pool_t[0:k, j:j + 1], fW1_t[j][0:k, :],
                                 start=(j == 0), stop=False)
            nc.tensor.matmul(z1p[:], ones32[:], fb_t["fb1"][:], start=False, stop=True)
            z1s = cst.tile([1, 128], f32)
            nc.scalar.activation(z1s[:], z1p[:], AF.Relu)
            nc.sync.dma_start(vec_b[:], z1s[:])
            z1T = cst.tile([128, 1], f32)
            nc.sync.dma_start(z1T[:], vec_b[0, :].rearrange("(p o) -> p o", o=1))
            z2p = pp2.tile([1, 64], f32, tag="z")
            nc.tensor.matmul(z2p[:], z1T[:], fW2_t[:], start=True, stop=False)
            nc.tensor.matmul(z2p[:], ones32[:], fb_t["fb2"][:], start=False, stop=True)
            z2s = cst.tile([1, 64], f32)
            nc.scalar.activation(z2s[:], z2p[:], AF.Relu)
            nc.sync.dma_start(vec_b[0:1, 0:64], z2s[:])
            z2T = cst.tile([64, 1], f32)
            nc.sync.dma_start(z2T[:], vec_b[0, 0:64].rearrange("(p o) -> p o", o=1))
            z3p = pp2.tile([1, 1], f32, tag="z")
            nc.tensor.matmul(z3p[:], z2T[:], fW3_t[:], start=True, stop=False)
            nc.tensor.matmul(z3p[:], ones32[:], fb_t["fb3"][:], start=False, stop=True)
            ys = cst.tile([1, 1], f32)
            nc.scalar.activation(ys[:], z3p[:], AF.Sigmoid)
            nc.sync.dma_start(y_ext[:], ys[:])

    nc.compile()
    return nc


def kernel(**inputs):
    g_meta = []
    for g, (s, d) in enumerate([("src1", "dst1"), ("src2", "dst2"), ("src3", "dst3")]):
        g_meta.append(_prep_graph(inputs[s], inputs[d]))
    nc = _build(g_meta)
    in_maps = []
    for c in range(NC_):
        m = {}
        for g, xn in enumerate(["x1", "x2", "x3"]):
            Ck, offs, drel, ind, outd, odf = g_meta[g]
            m[f"x{g}"] = np.asarray(inputs[xn], np.float32)
            m[f"off{g}"] = offs[c]
            m[f"dr{g}"] = drel[c]
            m[f"ind{g}"] = ind[c]
            m[f"outd{g}"] = outd[c]
            m[f"odf{g}"] = odf
        for nm in ["W1", "W2", "W3", "fW2"]:
            m[nm] = np.asarray(inputs[nm], np.float32)
        m["fW1"] = np.asarray(inputs["fW1"], np.float32)
        m["fW3"] = np.asarray(inputs["fW3"], np.float32).reshape(64, 1)
        for nm in ["b1", "b2", "b3", "fb1", "fb2", "fb3"]:
            m[nm] = np.asarray(inputs[nm], np.float32).reshape(1, -1)
        in_maps.append(m)
    res = run_bass_kernel_spmd(nc, in_maps, core_ids)
    globals()["LAST"] = res
    return np.asarray(res.results[0]["y"], np.float32).reshape(1)



# revision 7
# speedup vs baseline: 1.8940x; 1.8940x over previous
"""3-branch GCN (DGL GraphConv x3 + max-pool + MLP head) on 8 TRN2 NeuronCores.

Sharding: destination nodes (2500/core). L1's x[src] gather is a static
permutation of the input, so it is pre-gathered (and rsqrt(outdeg)-prescaled)
on the host and streamed with dense DMAs. L2/L3 gather h[src] rows from a
replicated DRAM table with one batched SWDGE dma_gather per dst tile
(amortizing the ~1us fixed descriptor-gen cost over ~2k rows), aggregate via
one-hot fp16 matmuls into PSUM, and apply the dense W matmul per dst tile.
Layer outputs are AllGathered; layers run graph-interleaved (layer-major) so
each AllGather hides under the other two graphs' compute. Max-pool is local +
a final AllReduce(max); the tiny MLP head runs replicated on every core.
"""
import numpy as np
import concourse.bass as bass
import concourse.bacc as bacc
import concourse.tile as tile
import concourse.mybir as mybir
from concourse import library_config
from concourse.bass_utils import run_bass_kernel_spmd

NC_ = 8
N = 20000
E = 320000
SH = N // NC_          # 2500 nodes per core
NT = 20                # dst tiles per core (19 full + 68-node partial)
D_IN, D_H = 128, 304
DPAD = 384             # fp16 row pad for 768B (256B-mult) rows
f16, f32 = mybir.dt.float16, mybir.dt.float32
i16 = mybir.dt.int16
AF = mybir.ActivationFunctionType
core_ids = list(range(NC_))


def _prep_graph(src, dst, x):
    """Per-core edge metadata + host pre-gathered L1 operand."""
    src = np.asarray(src).astype(np.int64)
    dst = np.asarray(dst).astype(np.int64)
    outdeg = np.bincount(src, minlength=N).clip(1).astype(np.float32)
    indeg = np.bincount(dst, minlength=N).clip(1).astype(np.float32)
    rso = (1.0 / np.sqrt(outdeg)).astype(np.float32)
    rsi = (1.0 / np.sqrt(indeg)).astype(np.float32)
    xs = (np.asarray(x, np.float32) * rso[:, None]).astype(np.float16)  # [N,128]
    per_core = []
    for c in range(NC_):
        m = (dst // SH) == c
        es, ed = src[m], dst[m] - c * SH
        tiles = []
        for t in range(NT):
            tm = (ed // 128) == t
            tiles.append((es[tm], ed[tm] - t * 128))
        per_core.append(tiles)
    # uniform chunk count per tile slot across cores (SPMD: one program)
    Ck = [max(int(np.ceil(len(per_core[c][t][0]) / 128)) for c in range(NC_)) or 1
          for t in range(NT)]
    nch = sum(Ck)
    offs = np.zeros((NC_, 128, nch), np.int64)             # pad -> row 0
    drel = np.full((NC_, 128, nch), -1.0, np.float32)      # -1 -> zero S row
    for c in range(NC_):
        j0 = 0
        for t in range(NT):
            es, er = per_core[c][t]
            npad = Ck[t] * 128
            e_s = np.zeros(npad, np.int64)
            e_r = np.full(npad, -1.0, np.float32)
            e_s[:len(es)] = es
            e_r[:len(er)] = er
            offs[c, :, j0:j0 + Ck[t]] = e_s.reshape(Ck[t], 128).T
            drel[c, :, j0:j0 + Ck[t]] = e_r.reshape(Ck[t], 128).T
            j0 += Ck[t]
    # wrapped int16 index layout for dma_gather: idx j at partition j%16,
    # col j//16, replicated across the 8 DSP-core groups of 16 partitions
    idx16 = np.zeros((NC_, 128, nch * 8), np.int16)
    xg = np.zeros((NC_, 128, nch * 128), np.float16)
    for c in range(NC_):
        j0 = 0
        for t in range(NT):
            ids = offs[c, :, j0:j0 + Ck[t]]                # [128, Ck] slot-major
            lin = ids.T.reshape(-1)                        # i = chunk*128 + slot
            w = lin.reshape(Ck[t] * 8, 16).T               # [16, Ck*8]
            idx16[c, :, j0 * 8:(j0 + Ck[t]) * 8] = np.tile(w, (8, 1))
            j0 += Ck[t]
        xg[c] = xs[offs[c]].reshape(128, nch * 128)
    # slot-ordered rsqrt(deg) arrays [128, NT] (pad rows -> 1.0)
    rin = np.ones((NC_, 128, NT), np.float32)
    rout = np.ones((NC_, 128, NT), np.float32)
    for c in range(NC_):
        for t in range(NT):
            lo = c * SH + t * 128
            hi = min(lo + 128, (c + 1) * SH)
            rin[c, :hi - lo, t] = rsi[lo:hi]
            rout[c, :hi - lo, t] = rso[lo:hi]
    return Ck, nch, drel, idx16, xg, rin, rout


def _build(g_meta):
    nc = bacc.Bacc(None, target_bir_lowering=False)
    ext = {}
    for g in range(3):
        Ck, nch = g_meta[g][0], g_meta[g][1]
        ext[f"xg{g}"] = nc.dram_tensor(f"xg{g}", [128, nch * 128], f16, kind="ExternalInput")
        ext[f"idx{g}"] = nc.dram_tensor(f"idx{g}", [128, nch * 8], i16, kind="ExternalInput")
        ext[f"dr{g}"] = nc.dram_tensor(f"dr{g}", [128, nch], f32, kind="ExternalInput")
        ext[f"rin{g}"] = nc.dram_tensor(f"rin{g}", [128, NT], f32, kind="ExternalInput")
        ext[f"rout{g}"] = nc.dram_tensor(f"rout{g}", [128, NT], f32, kind="ExternalInput")
    # W1 fp16; W2/W3 as 3 row-blocks of [128/49, 304] with bias folded in row 48
    ext["W1"] = nc.dram_tensor("W1", [D_IN, D_H], f16, kind="ExternalInput")
    ext["b1"] = nc.dram_tensor("b1", [1, D_H], f16, kind="ExternalInput")
    for L in (2, 3):
        ext[f"W{L}p"] = nc.dram_tensor(f"W{L}p", [3 * 128, D_H], f16, kind="ExternalInput")
    for nm, shp in [("fW1", [D_H, 128]), ("fb1", [1, 128]), ("fW2", [128, 64]),
                    ("fb2", [1, 64]), ("fW3", [64, 1]), ("fb3", [1, 1])]:
        ext[nm] = nc.dram_tensor(nm, shp, f32, kind="ExternalInput")
    y_ext = nc.dram_tensor("y", [1, 1], f32, kind="ExternalOutput")

    iota_d = nc.inline_tensor(np.tile(np.arange(128, dtype=np.float16), (128, 1)),
                              name="iota128")
    ident_d = nc.inline_tensor(np.eye(128, dtype=np.float32), name="ident")
    ones16_d = nc.inline_tensor(np.ones((1, 128), np.float16), name="ones16")
    ones32_d = nc.inline_tensor(np.ones((1, 1), np.float32), name="ones32")

    CKMAX = max(max(m[0]) for m in g_meta)

    with tile.TileContext(nc) as tc:
        with (
            tc.tile_pool(name="cst", bufs=1) as cst,
            tc.tile_pool(name="meta", bufs=1) as meta,
            tc.tile_pool(name="g", bufs=3) as gp,
            tc.tile_pool(name="s", bufs=4) as sp,
            tc.tile_pool(name="ps", bufs=2, space="PSUM") as pp,
            tc.tile_pool(name="ps2", bufs=2, space="PSUM") as pp2,
            tc.tile_pool(name="dram", bufs=1, space="DRAM") as dram,
        ):
            nc.gpsimd.load_library(library_config.mlp)

            iota_t = cst.tile([128, 128], f16)
            nc.sync.dma_start(iota_t[:], iota_d[:])
            ident_t = cst.tile([128, 128], f32)
            nc.sync.dma_start(ident_t[:], ident_d[:])
            ones16 = cst.tile([1, 128], f16)
            nc.sync.dma_start(ones16[:], ones16_d[:])
            ones32 = cst.tile([1, 1], f32)
            nc.sync.dma_start(ones32[:], ones32_d[:])

            # weights resident in SBUF
            w1t = cst.tile([128, D_H], f16, name="w1t")
            nc.sync.dma_start(w1t[:], ext["W1"][:])
            b1t = cst.tile([1, D_H], f16, name="b1t")
            nc.sync.dma_start(b1t[:], ext["b1"][:])
            W_t = {}
            for L in (2, 3):
                W_t[L] = []
                for j in range(3):
                    w = cst.tile([128, D_H], f16, name=f"w{L}_{j}")
                    k = 128 if j < 2 else 49
                    nc.sync.dma_start(w[0:k, :], ext[f"W{L}p"][j * 128:j * 128 + k, :])
                    W_t[L].append(w)
            fW1_t = []
            for j in range(3):
                k = 128 if j < 2 else 48
                w = cst.tile([128, 128], f32, name=f"fw1_{j}")
                nc.sync.dma_start(w[0:k, :], ext["fW1"][j * 128:j * 128 + k, :])
                fW1_t.append(w)
            fW2_t = cst.tile([128, 64], f32)
            nc.sync.dma_start(fW2_t[:], ext["fW2"][:])
            fW3_t = cst.tile([64, 1], f32)
            nc.sync.dma_start(fW3_t[:], ext["fW3"][:])
            fb_t = {}
            for nm, w in [("fb1", 128), ("fb2", 64), ("fb3", 1)]:
                b = cst.tile([1, w], f32, name=f"{nm}t")
                nc.sync.dma_start(b[:], ext[nm][:])
                fb_t[nm] = b

            # DRAM feature tables + per-(g,L) shard buffers
            hfA, hfB, sb1, sb2 = [], [], [], []
            for g in range(3):
                hfA.append(dram.tile([N, DPAD], f16, addr_space="Shared", name=f"hfA{g}"))
                hfB.append(dram.tile([N, DPAD], f16, addr_space="Shared", name=f"hfB{g}"))
                sb1.append(dram.tile([SH, DPAD], f16, name=f"sb1_{g}"))
                sb2.append(dram.tile([SH, DPAD], f16, name=f"sb2_{g}"))
            pool_in = dram.tile([128, 3], f32)
            pool_out = dram.tile([128, 3], f32, addr_space="Shared")
            vec_b = dram.tile([1, 128], f32)

            macc = cst.tile([128, D_H], f32)
            nc.vector.memset(macc[:], 0.0)

            # per-graph metadata resident in SBUF
            idx_sb, dr_sb, rin_sb, rout_sb = [], [], [], []
            for g in range(3):
                nch = g_meta[g][1]
                ix = meta.tile([128, nch * 8], i16, name=f"ix{g}")
                nc.sync.dma_start(ix[:], ext[f"idx{g}"][:])
                idx_sb.append(ix)
                d = meta.tile([128, nch], f32, name=f"drs{g}")
                nc.sync.dma_start(d[:], ext[f"dr{g}"][:])
                dr_sb.append(d)
                ri = meta.tile([128, NT], f32, name=f"ri{g}")
                nc.sync.dma_start(ri[:], ext[f"rin{g}"][:])
                rin_sb.append(ri)
                ro = meta.tile([128, NT], f32, name=f"ro{g}")
                nc.sync.dma_start(ro[:], ext[f"rout{g}"][:])
                rout_sb.append(ro)

            def agg_psum(g, j0, Ckt, DL, rhs_of):
                """One-hot aggregation of this tile's chunks into PSUM."""
                psum = pp.tile([128, D_H], f32, tag="agg")
                for c in range(Ckt):
                    s = sp.tile([128, 128], f16, tag="s")
                    nc.vector.tensor_scalar(
                        s[:], iota_t[:], dr_sb[g][:, j0 + c:j0 + c + 1],
                        None, mybir.AluOpType.is_equal)
                    nc.tensor.matmul(psum[:, 0:DL], s[:], rhs_of(c),
                                     start=(c == 0), stop=(c == Ckt - 1))
                return psum

            # ---- L1 (host pre-gathered operand, dense loads) ----
            for g in range(3):
                Ck = g_meta[g][0]
                j0 = 0
                for t in range(NT):
                    rows = 128 if t < NT - 1 else SH - (NT - 1) * 128
                    xgt = gp.tile([128, CKMAX * 128], f16, tag="xgt")
                    nc.sync.dma_start(xgt[:, 0:Ck[t] * 128],
                                      ext[f"xg{g}"][:, j0 * 128:(j0 + Ck[t]) * 128])
                    psum = agg_psum(g, j0, Ck[t], D_IN,
                                    lambda c: xgt[:, c * 128:(c + 1) * 128])
                    j0 += Ck[t]
                    zsb = gp.tile([128, D_IN], f32, tag="zsb1")
                    nc.scalar.activation(zsb[:], psum[:, 0:D_IN], AF.Copy,
                                         scale=rin_sb[g][:, t:t + 1])
                    tp = pp.tile([128, 128], f32, tag="tp")
                    nc.tensor.transpose(tp[:], zsb[:], ident_t[:])
                    at = gp.tile([128, 128], f16, tag="at")
                    nc.vector.tensor_copy(at[:], tp[:])
                    psum2 = pp2.tile([128, D_H], f32, tag="wout")
                    nc.tensor.matmul(psum2[:], at[:], w1t[:], start=True, stop=False)
                    nc.tensor.matmul(psum2[:], ones16[:], b1t[:], start=False, stop=True)
                    hsb = gp.tile([128, D_H], f16, tag="hsb")
                    nc.scalar.activation(hsb[:], psum2[:], AF.Relu,
                                         scale=rout_sb[g][:, t:t + 1])
                    nc.sync.dma_start(sb1[g][t * 128:t * 128 + rows, 0:D_H],
                                      hsb[0:rows, :])
                nc.gpsimd.collective_compute(
                    "AllGather", mybir.AluOpType.bypass, replica_groups=[core_ids],
                    ins=[sb1[g].opt()], outs=[hfA[g].opt()])

            # ---- L2 / L3 (batched dma_gather per dst tile) ----
            for L in (2, 3):
                src_tab = hfA if L == 2 else hfB
                for g in range(3):
                    Ck = g_meta[g][0]
                    j0 = 0
                    for t in range(NT):
                        rows = 128 if t < NT - 1 else SH - (NT - 1) * 128
                        gt = gp.tile([128, CKMAX, DPAD], f16, tag="gt")
                        # SWDGE ring holds 1024 descriptors -> <=8 chunks/gather
                        for c0 in range(0, Ck[t], 8):
                            c1 = min(Ck[t], c0 + 8)
                            nc.gpsimd.dma_gather(
                                gt[:, c0:c1, :], src_tab[g][:],
                                idx_sb[g][:, (j0 + c0) * 8:(j0 + c1) * 8],
                                (c1 - c0) * 128, (c1 - c0) * 128, DPAD)
                        psum = agg_psum(g, j0, Ck[t], D_H,
                                        lambda c: gt[:, c, 0:D_H])
                        j0 += Ck[t]
                        zsb = gp.tile([128, D_H + 1], f32, tag="zsb")
                        nc.scalar.activation(zsb[:, 0:D_H], psum[:, 0:D_H], AF.Copy,
                                             scale=rin_sb[g][:, t:t + 1])
                        nc.vector.memset(zsb[:, D_H:D_H + 1], 1.0)
                        psum2 = pp2.tile([128, D_H], f32, tag="wout")
                        for j in range(3):
                            k = 128 if j < 2 else 49
                            tp = pp.tile([128, 128], f32, tag="tp")
                            nc.tensor.transpose(tp[0:k, :], zsb[:, j * 128:j * 128 + k],
                                                ident_t[:])
                            at = gp.tile([128, 128], f16, tag="at")
                            nc.vector.tensor_copy(at[0:k, :], tp[0:k, :])
                            nc.tensor.matmul(psum2[:], at[0:k, :], W_t[L][j][0:k, :],
                                             start=(j == 0), stop=(j == 2))
                        if L == 2:
                            hsb = gp.tile([128, D_H], f16, tag="hsb")
                            nc.scalar.activation(hsb[:], psum2[:], AF.Relu,
                                                 scale=rout_sb[g][:, t:t + 1])
                            nc.sync.dma_start(sb2[g][t * 128:t * 128 + rows, 0:D_H],
                                              hsb[0:rows, :])
                        else:
                            hsb3 = gp.tile([128, D_H], f32, tag="hsb3")
                            nc.scalar.activation(hsb3[:], psum2[:], AF.Relu)
                            nc.vector.tensor_tensor(macc[0:rows, :], macc[0:rows, :],
                                                    hsb3[0:rows, :], mybir.AluOpType.max)
                    if L == 2:
                        nc.gpsimd.collective_compute(
                            "AllGather", mybir.AluOpType.bypass,
                            replica_groups=[core_ids],
                            ins=[sb2[g].opt()], outs=[hfB[g].opt()])

            # max over partitions via transpose + reduce, AllReduce, MLP
            pool_sb = cst.tile([128, 3], f32)
            for j in range(3):
                k = 128 if j < 2 else 48
                tp = pp.tile([128, 128], f32, tag="tp")
                nc.tensor.transpose(tp[0:k, :], macc[:, j * 128:j * 128 + k], ident_t[:])
                nc.vector.tensor_reduce(pool_sb[0:k, j:j + 1], tp[0:k, :],
                                        mybir.AxisListType.X, mybir.AluOpType.max)
            nc.sync.dma_start(pool_in[:], pool_sb[:])
            nc.gpsimd.collective_compute(
                "AllReduce", mybir.AluOpType.max, replica_groups=[core_ids],
                ins=[pool_in.opt()], outs=[pool_out.opt()])
            pool_t = cst.tile([128, 3], f32)
            nc.sync.dma_start(pool_t[:], pool_out[:])

            z1p = pp2.tile([1, 128], f32, tag="z")
            for j in range(3):
                k = 128 if j < 2 else 48
                nc.tensor.matmul(z1p[:], pool_t[0:k, j:j + 1], fW1_t[j][0:k, :],
                                 start=(j == 0), stop=False)
            nc.tensor.matmul(z1p[:], ones32[:], fb_t["fb1"][:], start=False, stop=True)
            z1s = cst.tile([1, 128], f32)
            nc.scalar.activation(z1s[:], z1p[:], AF.Relu)
            nc.sync.dma_start(vec_b[:], z1s[:])
            z1T = cst.tile([128, 1], f32)
            nc.sync.dma_start(z1T[:], vec_b[0, :].rearrange("(p o) -> p o", o=1))
            z2p = pp2.tile([1, 64], f32, tag="z")
            nc.tensor.matmul(z2p[:], z1T[:], fW2_t[:], start=True, stop=False)
            nc.tensor.matmul(z2p[:], ones32[:], fb_t["fb2"][:], start=False, stop=True)
            z2s = cst.tile([1, 64], f32)
            nc.scalar.activation(z2s[:], z2p[:], AF.Relu)
            nc.sync.dma_start(vec_b[0:1, 0:64], z2s[:])
            z2T = cst.tile([64, 1], f32)
            nc.sync.dma_start(z2T[:], vec_b[0, 0:64].rearrange("(p o) -> p o", o=1))
            z3p = pp2.tile([1, 1], f32, tag="z")
            nc.tensor.matmul(z3p[:], z2T[:], fW3_t[:], start=True, stop=False)
            nc.tensor.matmul(z3p[:], ones32[:], fb_t["fb3"][:], start=False, stop=True)
            ys = cst.tile([1, 1], f32)
            nc.scalar.activation(ys[:], z3p[:], AF.Sigmoid)
            nc.sync.dma_start(y_ext[:], ys[:])

    nc.compile()
    return nc


def kernel(**inputs):
    g_meta = []
    for g, (s, d, xn) in enumerate([("src1", "dst1", "x1"), ("src2", "dst2", "x2"),
                                    ("src3", "dst3", "x3")]):
        g_meta.append(_prep_graph(inputs[s], inputs[d], inputs[xn]))
    nc = _build(g_meta)
    # fold biases into W2/W3's third row-block (row 48 = bias; at's row 48 = 1)
    Wp = {}
    for L in (2, 3):
        W = np.asarray(inputs[f"W{L}"], np.float32)
        b = np.asarray(inputs[f"b{L}"], np.float32).reshape(-1)
        blk = np.zeros((3 * 128, D_H), np.float16)
        blk[0:128] = W[0:128]
        blk[128:256] = W[128:256]
        blk[256:256 + 48] = W[256:304]
        blk[256 + 48] = b
        Wp[L] = blk
    in_maps = []
    for c in range(NC_):
        m = {}
        for g in range(3):
            Ck, nch, drel, idx16, xg, rin, rout = g_meta[g]
            m[f"xg{g}"] = xg[c]
            m[f"idx{g}"] = idx16[c]
            m[f"dr{g}"] = drel[c]
            m[f"rin{g}"] = rin[c]
            m[f"rout{g}"] = rout[c]
        m["W1"] = np.asarray(inputs["W1"], np.float32).astype(np.float16)
        m["b1"] = np.asarray(inputs["b1"], np.float32).reshape(1, -1).astype(np.float16)
        m["W2p"] = Wp[2]
        m["W3p"] = Wp[3]
        m["fW1"] = np.asarray(inputs["fW1"], np.float32)
        m["fW2"] = np.asarray(inputs["fW2"], np.float32)
        m["fW3"] = np.asarray(inputs["fW3"], np.float32).reshape(64, 1)
        for nm in ["fb1", "fb2", "fb3"]:
            m[nm] = np.asarray(inputs[nm], np.float32).reshape(1, -1)
        in_maps.append(m)
    res = run_bass_kernel_spmd(nc, in_maps, core_ids)
    globals()["LAST"] = res
    return np.asarray(res.results[0]["y"], np.float32).reshape(1)


# revision 8
# speedup vs baseline: 2.6314x; 1.3894x over previous
"""3-branch GCN (DGL GraphConv x3 + max-pool + MLP head) on 8 TRN2 NeuronCores.

Sharding: destination nodes (2500/core). L1's x[src] gather is a static
permutation of the input, so it is pre-gathered (and rsqrt(outdeg)-prescaled)
on the host and streamed with dense DMAs. L2/L3 gather h[src] rows from a
replicated DRAM table with batched SWDGE dma_gathers (1024 rows each, round-
robined over the 4 SWDGE queues so all four DSP pairs generate descriptors
concurrently), aggregate via count-matrix fp16 matmuls into PSUM (per-tile
unique-src dedup folds edge multiplicity into S, built host-side and streamed
from DRAM), and apply the dense W matmul per dst tile. Layer outputs are
AllGathered; layers run graph-interleaved (layer-major) so each AllGather
hides under the other two graphs' compute. Max-pool is local + a final
AllReduce(max); the tiny MLP head runs replicated on every core.
"""
import numpy as np
import concourse.bass as bass
import concourse.bacc as bacc
import concourse.tile as tile
import concourse.mybir as mybir
from concourse import library_config
from concourse.bass_utils import run_bass_kernel_spmd

NC_ = 8
N = 20000
E = 320000
SH = N // NC_          # 2500 nodes per core
NT = 20                # dst tiles per core (19 full + 68-node partial)
D_IN, D_H = 128, 304
DPAD = 384             # fp16 row pad for 768B (256B-mult) rows
GQ = 8                 # chunks per dma_gather (1024 rows = SWDGE ring size)
f16, f32 = mybir.dt.float16, mybir.dt.float32
i16 = mybir.dt.int16
AF = mybir.ActivationFunctionType
core_ids = list(range(NC_))


def _prep_graph(src, dst, x):
    """Per-core deduped edge metadata, host-built S, pre-gathered L1 operand."""
    src = np.asarray(src).astype(np.int64)
    dst = np.asarray(dst).astype(np.int64)
    outdeg = np.bincount(src, minlength=N).clip(1).astype(np.float32)
    indeg = np.bincount(dst, minlength=N).clip(1).astype(np.float32)
    rso = (1.0 / np.sqrt(outdeg)).astype(np.float32)
    rsi = (1.0 / np.sqrt(indeg)).astype(np.float32)
    xs = (np.asarray(x, np.float32) * rso[:, None]).astype(np.float16)  # [N,128]
    # per (core, tile): unique srcs + count matrix columns
    uniqs = [[None] * NT for _ in range(NC_)]
    cnts = [[None] * NT for _ in range(NC_)]
    for c in range(NC_):
        m = (dst // SH) == c
        es, ed = src[m], dst[m] - c * SH
        for t in range(NT):
            tm = (ed // 128) == t
            u, inv = np.unique(es[tm], return_inverse=True)
            cm = np.zeros((max(len(u), 1), 128), np.float16)
            np.add.at(cm, (inv, ed[tm] - t * 128), 1.0)
            uniqs[c][t] = u if len(u) else np.zeros(1, np.int64)
            cnts[c][t] = cm
    Ck = [max(int(np.ceil(len(uniqs[c][t]) / 128)) for c in range(NC_)) or 1
          for t in range(NT)]
    nch = sum(Ck)
    S = np.zeros((NC_, 128, nch, 128), np.float16)   # [slot, chunk, dstrow]
    idx16 = np.zeros((NC_, 128, nch * 8), np.int16)
    xg = np.zeros((NC_, 128, nch * 128), np.float16)
    for c in range(NC_):
        j0 = 0
        for t in range(NT):
            u, cm = uniqs[c][t], cnts[c][t]
            npad = Ck[t] * 128
            up = np.zeros(npad, np.int64)
            up[:len(u)] = u
            cp = np.zeros((npad, 128), np.float16)
            cp[:len(u)] = cm
            # slot-major: slot i of chunk k = up[k*128 + i%... linear i = k*128+p
            S[c, :, j0:j0 + Ck[t], :] = cp.reshape(Ck[t], 128, 128).transpose(1, 0, 2)
            ids = up.reshape(Ck[t], 128).T            # [128, Ck]
            lin = ids.T.reshape(-1)
            w = lin.reshape(Ck[t] * 8, 16).T
            idx16[c, :, j0 * 8:(j0 + Ck[t]) * 8] = np.tile(w, (8, 1))
            xg[c, :, j0 * 128:(j0 + Ck[t]) * 128] = \
                xs[ids].reshape(128, Ck[t] * 128)
            j0 += Ck[t]
    rin = np.ones((NC_, 128, NT), np.float32)
    rout = np.ones((NC_, 128, NT), np.float32)
    for c in range(NC_):
        for t in range(NT):
            lo = c * SH + t * 128
            hi = min(lo + 128, (c + 1) * SH)
            rin[c, :hi - lo, t] = rsi[lo:hi]
            rout[c, :hi - lo, t] = rso[lo:hi]
    return Ck, nch, S.reshape(NC_, 128, nch * 128), idx16, xg, rin, rout


def _build(g_meta):
    nc = bacc.Bacc(None, target_bir_lowering=False, num_swdge_queues=4)
    ext = {}
    for g in range(3):
        nch = g_meta[g][1]
        ext[f"S{g}"] = nc.dram_tensor(f"S{g}", [128, nch * 128], f16, kind="ExternalInput")
        ext[f"xg{g}"] = nc.dram_tensor(f"xg{g}", [128, nch * 128], f16, kind="ExternalInput")
        ext[f"idx{g}"] = nc.dram_tensor(f"idx{g}", [128, nch * 8], i16, kind="ExternalInput")
        ext[f"rin{g}"] = nc.dram_tensor(f"rin{g}", [128, NT], f32, kind="ExternalInput")
        ext[f"rout{g}"] = nc.dram_tensor(f"rout{g}", [128, NT], f32, kind="ExternalInput")
    ext["W1"] = nc.dram_tensor("W1", [D_IN, D_H], f16, kind="ExternalInput")
    ext["b1"] = nc.dram_tensor("b1", [1, D_H], f16, kind="ExternalInput")
    for L in (2, 3):
        ext[f"W{L}p"] = nc.dram_tensor(f"W{L}p", [3 * 128, D_H], f16, kind="ExternalInput")
    for nm, shp in [("fW1", [D_H, 128]), ("fb1", [1, 128]), ("fW2", [128, 64]),
                    ("fb2", [1, 64]), ("fW3", [64, 1]), ("fb3", [1, 1])]:
        ext[nm] = nc.dram_tensor(nm, shp, f32, kind="ExternalInput")
    y_ext = nc.dram_tensor("y", [1, 1], f32, kind="ExternalOutput")

    ident_d = nc.inline_tensor(np.eye(128, dtype=np.float32), name="ident")
    ones16_d = nc.inline_tensor(np.ones((1, 128), np.float16), name="ones16")
    ones32_d = nc.inline_tensor(np.ones((1, 1), np.float32), name="ones32")

    with tile.TileContext(nc) as tc:
        with (
            tc.tile_pool(name="cst", bufs=1) as cst,
            tc.tile_pool(name="meta", bufs=1) as meta,
            tc.tile_pool(name="g", bufs=4) as gp,
            tc.tile_pool(name="s", bufs=4) as sp,
            tc.tile_pool(name="w", bufs=3) as wp,
            tc.tile_pool(name="ps", bufs=2, space="PSUM") as pp,
            tc.tile_pool(name="ps2", bufs=2, space="PSUM") as pp2,
            tc.tile_pool(name="dram", bufs=1, space="DRAM") as dram,
        ):
            nc.gpsimd.load_library(library_config.mlp)

            ident_t = cst.tile([128, 128], f32)
            nc.sync.dma_start(ident_t[:], ident_d[:])
            ones16 = cst.tile([1, 128], f16)
            nc.sync.dma_start(ones16[:], ones16_d[:])
            ones32 = cst.tile([1, 1], f32)
            nc.sync.dma_start(ones32[:], ones32_d[:])

            w1t = cst.tile([128, D_H], f16, name="w1t")
            nc.sync.dma_start(w1t[:], ext["W1"][:])
            b1t = cst.tile([1, D_H], f16, name="b1t")
            nc.sync.dma_start(b1t[:], ext["b1"][:])
            W_t = {}
            for L in (2, 3):
                W_t[L] = []
                for j in range(3):
                    w = cst.tile([128, D_H], f16, name=f"w{L}_{j}")
                    k = 128 if j < 2 else 49
                    nc.sync.dma_start(w[0:k, :], ext[f"W{L}p"][j * 128:j * 128 + k, :])
                    W_t[L].append(w)
            fW1_t = []
            for j in range(3):
                k = 128 if j < 2 else 48
                w = cst.tile([128, 128], f32, name=f"fw1_{j}")
                nc.sync.dma_start(w[0:k, :], ext["fW1"][j * 128:j * 128 + k, :])
                fW1_t.append(w)
            fW2_t = cst.tile([128, 64], f32)
            nc.sync.dma_start(fW2_t[:], ext["fW2"][:])
            fW3_t = cst.tile([64, 1], f32)
            nc.sync.dma_start(fW3_t[:], ext["fW3"][:])
            fb_t = {}
            for nm, w in [("fb1", 128), ("fb2", 64), ("fb3", 1)]:
                b = cst.tile([1, w], f32, name=f"{nm}t")
                nc.sync.dma_start(b[:], ext[nm][:])
                fb_t[nm] = b

            hfA, hfB, sb1, sb2 = [], [], [], []
            for g in range(3):
                hfA.append(dram.tile([N, DPAD], f16, addr_space="Shared", name=f"hfA{g}"))
                hfB.append(dram.tile([N, DPAD], f16, addr_space="Shared", name=f"hfB{g}"))
                sb1.append(dram.tile([SH, DPAD], f16, name=f"sb1_{g}"))
                sb2.append(dram.tile([SH, DPAD], f16, name=f"sb2_{g}"))
            pool_in = dram.tile([128, 3], f32)
            pool_out = dram.tile([128, 3], f32, addr_space="Shared")
            vec_b = dram.tile([1, 128], f32)

            macc = cst.tile([128, D_H], f32)
            nc.vector.memset(macc[:], 0.0)

            idx_sb, rin_sb, rout_sb = [], [], []
            for g in range(3):
                nch = g_meta[g][1]
                ix = meta.tile([128, nch * 8], i16, name=f"ix{g}")
                nc.sync.dma_start(ix[:], ext[f"idx{g}"][:])
                idx_sb.append(ix)
                ri = meta.tile([128, NT], f32, name=f"ri{g}")
                nc.sync.dma_start(ri[:], ext[f"rin{g}"][:])
                rin_sb.append(ri)
                ro = meta.tile([128, NT], f32, name=f"ro{g}")
                nc.sync.dma_start(ro[:], ext[f"rout{g}"][:])
                rout_sb.append(ro)

            def run_layer(L, g, qoff):
                """One GraphConv layer for graph g (1-indexed layer L)."""
                Ck, nch = g_meta[g][0], g_meta[g][1]
                DL = D_IN if L == 1 else D_H
                src_tab = None if L == 1 else (hfA[g] if L == 2 else hfB[g])
                # fetch one group of GQ chunks (S always; xg or dma_gather)
                cur = {}

                def fetch_group(gi):
                    g0 = gi * GQ
                    gc = min(GQ, nch - g0)
                    st = sp.tile([128, GQ * 128], f16, tag="st")
                    nc.scalar.dma_start(st[:, 0:gc * 128],
                                        ext[f"S{g}"][:, g0 * 128:(g0 + gc) * 128])
                    if L == 1:
                        gt = gp.tile([128, GQ * 128], f16, tag="xgt")
                        nc.sync.dma_start(gt[:, 0:gc * 128],
                                          ext[f"xg{g}"][:, g0 * 128:(g0 + gc) * 128])
                    else:
                        gt = gp.tile([128, GQ, DPAD], f16, tag="gt")
                        nc.gpsimd.dma_gather(
                            gt[:, 0:gc, :], src_tab[:],
                            idx_sb[g][:, g0 * 8:(g0 + gc) * 8],
                            gc * 128, gc * 128, DPAD,
                            queue_num=(gi + qoff) % 4)
                    cur["gi"], cur["st"], cur["gt"] = gi, st, gt

                fetch_group(0)
                j0 = 0
                for t in range(NT):
                    rows = 128 if t < NT - 1 else SH - (NT - 1) * 128
                    psum = pp.tile([128, D_H], f32, tag="agg")
                    for jc in range(Ck[t]):
                        j = j0 + jc
                        gi, sl = j // GQ, j % GQ
                        if gi != cur["gi"]:
                            fetch_group(gi)
                        st, gt = cur["st"], cur["gt"]
                        rhs = (gt[:, sl * 128:(sl + 1) * 128] if L == 1
                               else gt[:, sl, 0:D_H])
                        nc.tensor.matmul(psum[:, 0:DL],
                                         st[:, sl * 128:(sl + 1) * 128], rhs,
                                         start=(jc == 0), stop=(jc == Ck[t] - 1))
                    j0 += Ck[t]
                    psum2 = pp2.tile([128, D_H], f32, tag="wout")
                    if L == 1:
                        zsb = gp.tile([128, D_IN], f32, tag="zsb1")
                        nc.scalar.activation(zsb[:], psum[:, 0:D_IN], AF.Copy,
                                             scale=rin_sb[g][:, t:t + 1])
                        tp = pp.tile([128, 128], f32, tag="tp")
                        nc.tensor.transpose(tp[:], zsb[:], ident_t[:])
                        at = gp.tile([128, 128], f16, tag="at")
                        nc.vector.tensor_copy(at[:], tp[:])
                        nc.tensor.matmul(psum2[:], at[:], w1t[:], start=True, stop=False)
                        nc.tensor.matmul(psum2[:], ones16[:], b1t[:],
                                         start=False, stop=True)
                    else:
                        zsb = gp.tile([128, D_H + 1], f32, tag="zsb")
                        nc.scalar.activation(zsb[:, 0:D_H], psum[:, 0:D_H], AF.Copy,
                                             scale=rin_sb[g][:, t:t + 1])
                        nc.vector.memset(zsb[:, D_H:D_H + 1], 1.0)
                        for j in range(3):
                            k = 128 if j < 2 else 49
                            tp = pp.tile([128, 128], f32, tag="tp")
                            nc.tensor.transpose(tp[0:k, :],
                                                zsb[:, j * 128:j * 128 + k], ident_t[:])
                            at = gp.tile([128, 128], f16, tag="at")
                            nc.vector.tensor_copy(at[0:k, :], tp[0:k, :])
                            nc.tensor.matmul(psum2[:], at[0:k, :], W_t[L][j][0:k, :],
                                             start=(j == 0), stop=(j == 2))
                    if L < 3:
                        dst_sb = sb1[g] if L == 1 else sb2[g]
                        hsb = gp.tile([128, D_H], f16, tag="hsb")
                        nc.scalar.activation(hsb[:], psum2[:], AF.Relu,
                                             scale=rout_sb[g][:, t:t + 1])
                        nc.sync.dma_start(dst_sb[t * 128:t * 128 + rows, 0:D_H],
                                          hsb[0:rows, :])
                    else:
                        hsb3 = gp.tile([128, D_H], f32, tag="hsb3")
                        nc.scalar.activation(hsb3[:], psum2[:], AF.Relu)
                        nc.vector.tensor_tensor(macc[0:rows, :], macc[0:rows, :],
                                                hsb3[0:rows, :], mybir.AluOpType.max)
                if L < 3:
                    nc.gpsimd.collective_compute(
                        "AllGather", mybir.AluOpType.bypass, replica_groups=[core_ids],
                        ins=[(sb1[g] if L == 1 else sb2[g]).opt()],
                        outs=[(hfA[g] if L == 1 else hfB[g]).opt()])

            qoff = 0
            for L in (1, 2, 3):
                for g in range(3):
                    run_layer(L, g, qoff)
                    qoff += 1

            # max over partitions via transpose + reduce, AllReduce, MLP
            pool_sb = cst.tile([128, 3], f32)
            for j in range(3):
                k = 128 if j < 2 else 48
                tp = pp.tile([128, 128], f32, tag="tp")
                nc.tensor.transpose(tp[0:k, :], macc[:, j * 128:j * 128 + k], ident_t[:])
                nc.vector.tensor_reduce(pool_sb[0:k, j:j + 1], tp[0:k, :],
                                        mybir.AxisListType.X, mybir.AluOpType.max)
            nc.sync.dma_start(pool_in[:], pool_sb[:])
            nc.gpsimd.collective_compute(
                "AllReduce", mybir.AluOpType.max, replica_groups=[core_ids],
                ins=[pool_in.opt()], outs=[pool_out.opt()])
            pool_t = cst.tile([128, 3], f32)
            nc.sync.dma_start(pool_t[:], pool_out[:])

            z1p = pp2.tile([1, 128], f32, tag="z")
            for j in range(3):
                k = 128 if j < 2 else 48
                nc.tensor.matmul(z1p[:], pool_t[0:k, j:j + 1], fW1_t[j][0:k, :],
                                 start=(j == 0), stop=False)
            nc.tensor.matmul(z1p[:], ones32[:], fb_t["fb1"][:], start=False, stop=True)
            z1s = cst.tile([1, 128], f32)
            nc.scalar.activation(z1s[:], z1p[:], AF.Relu)
            nc.sync.dma_start(vec_b[:], z1s[:])
            z1T = cst.tile([128, 1], f32)
            nc.sync.dma_start(z1T[:], vec_b[0, :].rearrange("(p o) -> p o", o=1))
            z2p = pp2.tile([1, 64], f32, tag="z")
            nc.tensor.matmul(z2p[:], z1T[:], fW2_t[:], start=True, stop=False)
            nc.tensor.matmul(z2p[:], ones32[:], fb_t["fb2"][:], start=False, stop=True)
            z2s = cst.tile([1, 64], f32)
            nc.scalar.activation(z2s[:], z2p[:], AF.Relu)
            nc.sync.dma_start(vec_b[0:1, 0:64], z2s[:])
            z2T = cst.tile([64, 1], f32)
            nc.sync.dma_start(z2T[:], vec_b[0, 0:64].rearrange("(p o) -> p o", o=1))
            z3p = pp2.tile([1, 1], f32, tag="z")
            nc.tensor.matmul(z3p[:], z2T[:], fW3_t[:], start=True, stop=False)
            nc.tensor.matmul(z3p[:], ones32[:], fb_t["fb3"][:], start=False, stop=True)
            ys = cst.tile([1, 1], f32)
            nc.scalar.activation(ys[:], z3p[:], AF.Sigmoid)
            nc.sync.dma_start(y_ext[:], ys[:])

    nc.compile()
    return nc


def kernel(**inputs):
    g_meta = []
    for g, (s, d, xn) in enumerate([("src1", "dst1", "x1"), ("src2", "dst2", "x2"),
                                    ("src3", "dst3", "x3")]):
        g_meta.append(_prep_graph(inputs[s], inputs[d], inputs[xn]))
    nc = _build(g_meta)
    # fold biases into W2/W3's third row-block (row 48 = bias; at's row 48 = 1)
    Wp = {}
    for L in (2, 3):
        W = np.asarray(inputs[f"W{L}"], np.float32)
        b = np.asarray(inputs[f"b{L}"], np.float32).reshape(-1)
        blk = np.zeros((3 * 128, D_H), np.float16)
        blk[0:128] = W[0:128]
        blk[128:256] = W[128:256]
        blk[256:256 + 48] = W[256:304]
        blk[256 + 48] = b
        Wp[L] = blk
    in_maps = []
    for c in range(NC_):
        m = {}
        for g in range(3):
            Ck, nch, S, idx16, xg, rin, rout = g_meta[g]
            m[f"S{g}"] = S[c]
            m[f"xg{g}"] = xg[c]
            m[f"idx{g}"] = idx16[c]
            m[f"rin{g}"] = rin[c]
            m[f"rout{g}"] = rout[c]
        m["W1"] = np.asarray(inputs["W1"], np.float32).astype(np.float16)
        m["b1"] = np.asarray(inputs["b1"], np.float32).reshape(1, -1).astype(np.float16)
        m["W2p"] = Wp[2]
        m["W3p"] = Wp[3]
        m["fW1"] = np.asarray(inputs["fW1"], np.float32)
        m["fW2"] = np.asarray(inputs["fW2"], np.float32)
        m["fW3"] = np.asarray(inputs["fW3"], np.float32).reshape(64, 1)
        for nm in ["fb1", "fb2", "fb3"]:
            m[nm] = np.asarray(inputs[nm], np.float32).reshape(1, -1)
        in_maps.append(m)
    res = run_bass_kernel_spmd(nc, in_maps, core_ids)
    globals()["LAST"] = res
    return np.asarray(res.results[0]["y"], np.float32).reshape(1)
